# revision 1
# baseline (speedup 1.0000x reference)
"""Multi-head causal self-attention on 8 Trainium2 NeuronCores (Bass/Tile).

Problem: y = proj(softmax(causal_mask(Q K^T / sqrt(D))) V) for B=2, T=2048,
C=1024, H=16 heads, D=64.

Sharding (tensor-parallel over heads, 8-way):
  - Core i owns heads {2i, 2i+1}: computes qT/kT/vT for its heads over both
    batches (full x, its 128-column slice of Wqkv) and runs causal attention
    per head fully on-core, producing normalized yT_local (head-dims on
    partitions, time on the free axis).
  - Two 8-way AllToAlls (one per local head-row; the first overlaps the
    second head's compute) reshard head-split -> time-split: core j ends
    with yT_full [1024, 512] for time-slice j of the flattened (B*T) axis
    and computes its [512, 1024] slice of y @ Wproj.
  - The host concatenates the 8 time-slices into [2, 2048, 1024].

Matmuls use the float32r PE path (fp32 rounded to 11 explicit mantissa
bits; measured 227 ns per 512-wide matmul back-to-back — same rate as
bf16 — with fp32 accumulation in PSUM). fp32r operands must carry the
float32r dtype from their producer: host tensors are pre-rounded and
DMA'd into float32r tiles; on-chip producers write float32r directly.

Attention is computed transposed (S^T[k, q], keys on partitions): no
transposes anywhere in the attention path, exp on ScalarE straight out of
PSUM, and the softmax denominator comes free from a ones column appended
to V (row 64 of the P@V accumulator). Causality is exact: S^T blocks
strictly above the diagonal are skipped, diagonal blocks use a restricted
column range plus a triangular multiplicative mask after exp. Full blocks
are computed in 1024-wide pairs so one ACTIVATE covers two blocks.

The kernel is emitted with interleaved instruction streams (generators):
batch-1 projections are woven into head-0 attention and the output
projection into head-1 attention, so the in-order PE queue always has
independent matmuls to run while ScalarE works through exp. Softmax
normalization is deferred per (head, batch): denominators are DMA-packed
into a [16, 128] tile, inverted with one wide reciprocal, and broadcast
across partitions with K=1 PE matmuls, keeping slow reciprocals off the
PE critical path.
"""

import numpy as np

import concourse.bass as bass
import concourse.mybir as mybir
import concourse.tile as tile
from concourse import bacc
from concourse import bass_utils

F32 = mybir.dt.float32
F32R = mybir.dt.float32r
AF = mybir.ActivationFunctionType

B, T, C = 2, 2048, 1024
H, D = 16, 64
N_CORES = 8
HL = H // N_CORES        # heads per core = 2
NCT = C // 128           # contraction tiles = 8
NQ = T // 512            # q tiles per batch = 4
NK = T // 128            # k tiles per batch = 16
SCALE = 1.0 / float(np.sqrt(D))  # 0.125

_BUILD_CACHE = {}


def round_f32r(x):
    """fp32 -> fp32r rounding (11 explicit mantissa bits, nearest-even)."""
    u = np.asarray(x, np.float32).view(np.uint32).astype(np.uint64)
    low = u & np.uint64(0xFFF)
    base = u & np.uint64(0xFFFFF000)
    lsb = (u >> np.uint64(12)) & np.uint64(1)
    round_up = (low > 0x800) | ((low == 0x800) & (lsb == 1))
    out = base + np.where(round_up, np.uint64(0x1000), np.uint64(0))
    return (out & np.uint64(0xFFFFFFFF)).astype(np.uint32).view(np.float32)


def _drain(*gens):
    """Round-robin the generators until all are exhausted."""
    active = list(gens)
    while active:
        nxt = []
        for g in active:
            try:
                next(g)
                nxt.append(g)
            except StopIteration:
                pass
        active = nxt


def build_kernel(apply_pad_mask: bool):
    nc = bacc.Bacc(
        "TRN2", target_bir_lowering=False, debug=False, num_devices=N_CORES
    )
    xT = nc.dram_tensor("xT", [C, B * T], F32R, kind="ExternalInput").ap()
    wqkv = nc.dram_tensor("wqkv", [C, 3 * HL * D], F32R, kind="ExternalInput").ap()
    wo = nc.dram_tensor("wo", [C, C], F32R, kind="ExternalInput").ap()
    tri = nc.dram_tensor("tri", [128, 128], F32, kind="ExternalInput").ap()
    ident = nc.dram_tensor("ident", [128, 128], F32R, kind="ExternalInput").ap()
    padk = nc.dram_tensor("padk", [128, B * NK], F32, kind="ExternalInput").ap()
    out = nc.dram_tensor("out", [512, C], F32, kind="ExternalOutput").ap()

    with tile.TileContext(nc) as tc:
        with (
            tc.tile_pool(name="const", bufs=1) as constp,
            tc.tile_pool(name="qk", bufs=1) as qkp,
            tc.tile_pool(name="vv", bufs=1) as vvp,
            tc.tile_pool(name="xw", bufs=1) as xwp,
            tc.tile_pool(name="work", bufs=2) as wk,
            tc.tile_pool(name="ytmp_pool", bufs=2) as ytp,
            tc.tile_pool(name="ps_ss", bufs=2, space="PSUM") as ps_ss,
            tc.tile_pool(name="ps_main", bufs=2, space="PSUM") as ps_main,
            tc.tile_pool(name="ps_y", bufs=2, space="PSUM") as ps_y,
            tc.tile_pool(name="dram", bufs=1, space="DRAM") as dram,
        ):
            # ---------------- constants ----------------
            tri_sb = constp.tile([128, 128], F32, name="tri_sb")
            nc.sync.dma_start(tri_sb[:], tri[:])
            id_sb = constp.tile([128, 128], F32R, name="id_sb")
            nc.sync.dma_start(id_sb[:], ident[:])
            ones_f = constp.tile([65, 64], F32, name="ones_f")
            ones_sb = constp.tile([65, 64], F32R, name="ones_sb")
            for r in (0, 32, 64):
                nc.vector.memset(ones_f[r:r + 1, :], 1.0)
                nc.gpsimd.tensor_copy(ones_sb[r:r + 1, :], ones_f[r:r + 1, :])
            onesc_f = constp.tile([128, HL], F32, name="onesc_f")
            nc.vector.memset(onesc_f[:], 1.0)
            onesc = constp.tile([128, HL], F32R, name="onesc")
            nc.gpsimd.tensor_copy(onesc[:], onesc_f[:])
            if apply_pad_mask:
                padk_sb = constp.tile([128, B * NK], F32, name="padk_sb")
                nc.sync.dma_start(padk_sb[:], padk[:])

            a2a_in = [dram.tile([N_CORES, 64, 512], F32R, name=f"a2a_in{h}")
                      for h in range(HL)]
            a2a_out = [dram.tile([N_CORES, 64, 512], F32R, name=f"a2a_out{h}")
                       for h in range(HL)]

            # weights first so the first matmul group is ready ASAP
            wqkv_sb = []
            for ct in range(NCT):
                w_sb = xwp.tile([128, 3 * HL * D], F32R, name=f"wqkv{ct}",
                                tag=f"wqkv{ct}")
                nc.sync.dma_start(w_sb[:], wqkv[ct * 128:(ct + 1) * 128, :])
                wqkv_sb.append(w_sb)

            qT = [None] * B
            kT = [None] * B
            V = [[None] * NK for _ in range(B)]
            ytn = [[None] * (B * NQ) for _ in range(HL)]

            def qkv_emit(b):
                """Projections for batch b: yields between schedulable
                chunks so the PE stream can interleave with attention."""
                xt_sb = []
                for half in range(2):
                    for ct in range(NCT):
                        if half == 0:
                            x_sb = xwp.tile([128, T], F32R, name=f"xt{ct}",
                                            tag=f"xt{ct}")
                            xt_sb.append(x_sb)
                        nc.sync.dma_start(
                            xt_sb[ct][:, half * 1024:(half + 1) * 1024],
                            xT[ct * 128:(ct + 1) * 128,
                               b * T + half * 1024:b * T + (half + 1) * 1024],
                        )
                qT[b] = qkp.tile([128, T], F32R, name="qT", tag=f"qT{b}")
                kT[b] = qkp.tile([128, T], F32R, name="kT", tag=f"kT{b}")
                vT = qkp.tile([128, T], F32R, name="vT", tag="vT")
                for which, dst in ((2, vT), (1, kT[b]), (0, qT[b])):
                    for n in range(NQ):
                        p = ps_main.tile([128, 512], F32, name="p_mm",
                                         tag="ps")
                        for ct in range(NCT):
                            nc.tensor.matmul(
                                p[:],
                                wqkv_sb[ct][:, which * 128:(which + 1) * 128],
                                xt_sb[ct][:, n * 512:(n + 1) * 512],
                                start=(ct == 0),
                                stop=(ct == NCT - 1),
                            )
                        nc.vector.tensor_copy(dst[:, n * 512:(n + 1) * 512],
                                              p[:])
                        yield
                for kt in range(NK):
                    v_sb = vvp.tile([128, HL * 65], F32R, name=f"V{b}_{kt}",
                                    tag=f"V{b}_{kt}")
                    pt = ps_main.tile([128, 128], F32R, name="p_tr", tag="ps")
                    nc.tensor.transpose(pt[:], vT[:, kt * 128:(kt + 1) * 128],
                                        id_sb[:])
                    v3 = v_sb[:].rearrange("p (h e) -> p h e", h=HL)
                    nc.gpsimd.tensor_copy(v3[:, :, 64], onesc[:])
                    nc.vector.tensor_copy(
                        v3[:, :, 0:64],
                        pt[:].rearrange("p (h e) -> p h e", h=HL),
                    )
                    V[b][kt] = v_sb
                    if kt % 4 == 3:
                        yield

            def attn_emit(h, b):
                """Attention for head-row h, batch b. Yields per exp-block."""
                h0 = h * 64
                coll = wk.tile([4 * NQ, 128], F32, name="coll",
                               tag="coll", bufs=3)
                for j in range(NQ):
                    q0 = j * 512
                    py = ps_y.tile([65, 512], F32, name="p_y", tag="py")
                    n_kt = 4 * j + 4
                    # paired full blocks, then restricted diagonal singles
                    chunks = []
                    kt = 0
                    while kt < 4 * j:
                        chunks.append((kt, kt + 1))
                        kt += 2
                    for kt in range(4 * j, n_kt):
                        chunks.append((kt,))
                    for chunk in chunks:
                        pss = ps_ss.tile([128, 1024], F32, name="p_s",
                                         tag="pss")
                        lo = None
                        for ci, kt in enumerate(chunk):
                            i = kt - 4 * j
                            off = 128 * i if i >= 0 else 0
                            base = 512 * ci
                            if lo is None:
                                lo = base + off
                            nc.tensor.matmul(
                                pss[:, base + off:base + 512],
                                kT[b][h0:h0 + 64, kt * 128:(kt + 1) * 128],
                                qT[b][h0:h0 + 64, q0 + off:q0 + 512],
                                start=True,
                                stop=True,
                            )
                        hi = 512 * (len(chunk) - 1) + 512
                        p_sb = wk.tile([128, 1024], F32R, name="p_sb",
                                       tag="p_sb", bufs=3)
                        nc.scalar.activation(
                            p_sb[:, lo:hi], pss[:, lo:hi], AF.Exp,
                            scale=float(SCALE),
                        )
                        for ci, kt in enumerate(chunk):
                            i = kt - 4 * j
                            off = 128 * i if i >= 0 else 0
                            base = 512 * ci
                            if i >= 0:
                                nc.vector.tensor_mul(
                                    p_sb[:, base + off:base + off + 128],
                                    p_sb[:, base + off:base + off + 128],
                                    tri_sb[:],
                                )
                            if apply_pad_mask:
                                nc.vector.tensor_scalar_mul(
                                    p_sb[:, base + off:base + 512],
                                    p_sb[:, base + off:base + 512],
                                    padk_sb[:, b * NK + kt:b * NK + kt + 1],
                                )
                            nc.tensor.matmul(
                                py[0:65, off:512],
                                V[b][kt][:, h * 65:(h + 1) * 65],
                                p_sb[:, base + off:base + 512],
                                start=(kt == 0),
                                stop=(kt == n_kt - 1),
                            )
                        yield
                    # evacuate PV accumulator
                    m = b * NQ + j
                    yu = ytp.tile([64, 512], F32R, name="ytn",
                                  tag=f"ytn{m}", bufs=1)
                    nc.vector.tensor_copy(yu[:], py[0:64, :])
                    ytn[h][m] = yu
                    srow = wk.tile([65, 512], F32, name="srow", tag="srow",
                                   bufs=4)
                    nc.vector.tensor_copy(srow[64:65, :], py[64:65, :])
                    nc.sync.dma_start(coll[4 * j:4 * j + 4, :],
                                      srow[64:65, :])
                # wide reciprocal for this (h, b): all lanes busy
                rcol = wk.tile([4 * NQ, 128], F32R, name="rcol", tag="rcol",
                               bufs=3)
                with nc.allow_low_precision(reason="fp32r softmax denom"):
                    nc.vector.reciprocal(rcol[:], coll[:])
                for j in range(NQ):
                    m = b * NQ + j
                    rbase = 32 * (j % 3)
                    rr = wk.tile([65, 512], F32R, name="rrow",
                                 tag=f"rr{j // 3}", bufs=2)
                    nc.sync.dma_start(rr[rbase:rbase + 1, :],
                                      rcol[4 * j:4 * j + 4, :])
                    pb = ps_main.tile([64, 512], F32, name="p_b", tag="ps")
                    nc.tensor.matmul(
                        pb[:], ones_sb[rbase:rbase + 1, :],
                        rr[rbase:rbase + 1, :], start=True, stop=True,
                    )
                    nc.vector.tensor_mul(ytn[h][m][:], ytn[h][m][:], pb[:])
                    nc.sync.dma_start(a2a_in[h][m, :, :], ytn[h][m][:])
                    yield

            wo_sb = []
            ytf = []

            def wo_ytf0_emit():
                # prefetch Wproj into the (now dead) x slots and pull the
                # h=0 halves of yT_full as soon as AllToAll #1 lands
                for ct in range(NCT):
                    w_sb = xwp.tile([128, C], F32R, name=f"wo{ct}",
                                    tag=f"xt{ct}")
                    nc.sync.dma_start(w_sb[:], wo[ct * 128:(ct + 1) * 128, :])
                    wo_sb.append(w_sb)
                    yield
                for s in range(N_CORES):
                    y_sb = xwp.tile([128, 512], F32R, name=f"ytf{s}",
                                    tag=f"wqkv{s}")
                    nc.sync.dma_start(y_sb[0:64, :], a2a_out[0][s, :, :])
                    ytf.append(y_sb)
                    yield

            def proj_emit():
                for s in range(N_CORES):
                    nc.sync.dma_start(ytf[s][64:128, :], a2a_out[1][s, :, :])
                yield
                for mt in range(4):
                    o_sb = wk.tile([128, C], F32, name="o_sb", tag="o_sb")
                    for n in range(2):
                        po = ps_main.tile([128, 512], F32, name="p_o",
                                          tag="ps")
                        for ct in range(NCT):
                            nc.tensor.matmul(
                                po[:],
                                ytf[ct][:, mt * 128:(mt + 1) * 128],
                                wo_sb[ct][:, n * 512:(n + 1) * 512],
                                start=(ct == 0),
                                stop=(ct == NCT - 1),
                            )
                        nc.vector.tensor_copy(o_sb[:, n * 512:(n + 1) * 512],
                                              po[:])
                        yield
                    nc.sync.dma_start(out[mt * 128:(mt + 1) * 128, :],
                                      o_sb[:])

            # ---------------- emission schedule ----------------
            _drain(qkv_emit(0))
            _drain(attn_emit(0, 0), qkv_emit(1))
            _drain(attn_emit(0, 1))
            nc.gpsimd.collective_compute(
                "AllToAll", mybir.AluOpType.bypass,
                replica_groups=[list(range(N_CORES))],
                ins=[a2a_in[0].opt()], outs=[a2a_out[0].opt()],
            )
            _drain(attn_emit(1, 0), wo_ytf0_emit())
            _drain(attn_emit(1, 1))
            nc.gpsimd.collective_compute(
                "AllToAll", mybir.AluOpType.bypass,
                replica_groups=[list(range(N_CORES))],
                ins=[a2a_in[1].opt()], outs=[a2a_out[1].opt()],
            )
            _drain(proj_emit())

    nc.compile()
    return nc


def _host_inputs(x, tok_mask, Wqkv, Wproj, apply_pad_mask):
    x = np.ascontiguousarray(np.asarray(x, dtype=np.float32))
    Wqkv = np.ascontiguousarray(np.asarray(Wqkv, dtype=np.float32))
    Wproj = np.ascontiguousarray(np.asarray(Wproj, dtype=np.float32))
    xT = round_f32r(np.concatenate([x[b].T for b in range(B)], axis=1))
    wo_r = round_f32r(Wproj)
    r = np.arange(128)
    tri = (r[None, :] >= r[:, None]).astype(np.float32)  # keep if col >= row
    ident = np.eye(128, dtype=np.float32)
    if apply_pad_mask:
        padk = np.zeros((128, B * NK), np.float32)
        for b in range(B):
            padk[:, b * NK:(b + 1) * NK] = (
                np.asarray(tok_mask[b]).reshape(NK, 128).T.astype(np.float32)
            )
    else:
        padk = np.ones((128, B * NK), np.float32)

    in_maps = []
    for core in range(N_CORES):
        cols = slice(core * HL * D, (core + 1) * HL * D)
        wqkv_c = round_f32r(
            np.concatenate(
                [Wqkv[:, :C][:, cols], Wqkv[:, C:2 * C][:, cols],
                 Wqkv[:, 2 * C:][:, cols]],
                axis=1,
            )
        )
        in_maps.append(
            {
                "xT": xT,
                "wqkv": wqkv_c,
                "wo": wo_r,
                "tri": tri,
                "ident": ident,
                "padk": padk,
            }
        )
    return in_maps


def kernel(x, tok_mask, Wqkv, Wproj, _run_kwargs=None):
    tok = np.asarray(tok_mask)
    apply_pad_mask = not bool(tok.all())
    key = apply_pad_mask
    if key not in _BUILD_CACHE:
        _BUILD_CACHE[key] = build_kernel(apply_pad_mask)
    nc = _BUILD_CACHE[key]
    in_maps = _host_inputs(x, tok_mask, Wqkv, Wproj, apply_pad_mask)
    kw = dict(_run_kwargs or {})
    res = bass_utils.run_bass_kernel_spmd(
        nc, in_maps, core_ids=list(range(N_CORES)), **kw
    )
    out = np.empty((B, T, C), np.float32)
    for core in range(N_CORES):
        b, jj = divmod(core, NQ)
        out[b, jj * 512:(jj + 1) * 512, :] = res.results[core]["out"]
    kernel.last_result = res
    return out



# revision 3
# speedup vs baseline: 1.1783x; 1.1783x over previous
"""Multi-head causal self-attention on 8 Trainium2 NeuronCores (Bass/Tile).

Problem: y = proj(softmax(causal_mask(Q K^T / sqrt(D))) V) for B=2, T=2048,
C=1024, H=16 heads, D=64.

Sharding (tensor-parallel over heads, 8-way):
  - Core i owns heads {2i, 2i+1}: computes qT/kT/vT for its heads over both
    batches (full x, its 128-column slice of Wqkv) and runs causal attention
    per head fully on-core, producing normalized yT_local (head-dims on
    partitions, time on the free axis).
  - One 8-way AllToAll per batch reshards head-split -> time-split: core j
    ends with ytf[b] tiles [128, 256] covering time cols [256j, 256j+256)
    of batch b for all heads, and computes out rows = [b0 slice; b1 slice]
    via y @ Wproj.  Host concatenates the 8 col-slices per batch.

Processing is batch-major: attn(b=0) for BOTH heads (their K=64 S^T matmuls
land in different PE row-groups and run concurrently), then the b=0
AllToAll overlaps attn(b=1), and proj(b=0) overlaps the b=1 AllToAll, so
only a 0.5MB collective + half the projection remain on the tail.

All tensors are bf16 (fp32 PSUM accumulation); the softmax denominator
path (reciprocal + K=1 broadcast matmuls) stays fp32/fp32r.  Attention is
computed transposed (S^T[k, q], keys on partitions): no transposes in the
attention path, exp on ScalarE straight out of PSUM, and the denominator
comes free from a ones column appended to V (row 64 of the P@V
accumulator).  Causality is exact: S^T blocks strictly above the diagonal
are skipped, diagonal blocks use a restricted column range plus a
triangular multiplicative mask after exp.  Full blocks are computed in
1024-wide pairs so one ACTIVATE covers two blocks.

The kernel is emitted with interleaved instruction streams (generators):
batch-1 projections are woven into batch-0 attention and the b=0 output
projection into batch-1 attention, so the in-order PE queue always has
independent matmuls to run while ScalarE works through exp.
"""

import numpy as np
import ml_dtypes

import concourse.bass as bass
import concourse.mybir as mybir
import concourse.tile as tile
from concourse import bacc
from concourse import bass_utils

F32 = mybir.dt.float32
F32R = mybir.dt.float32r
BF16 = mybir.dt.bfloat16
AF = mybir.ActivationFunctionType

B, T, C = 2, 2048, 1024
H, D = 16, 64
N_CORES = 8
HL = H // N_CORES        # heads per core = 2
NCT = C // 128           # contraction tiles = 8
NQ = T // 512            # q tiles per batch = 4
NK = T // 128            # k tiles per batch = 16
SCALE = 1.0 / float(np.sqrt(D))  # 0.125

_BUILD_CACHE = {}


def _drain(*gens):
    """Round-robin the generators until all are exhausted."""
    active = list(gens)
    while active:
        nxt = []
        for g in active:
            try:
                next(g)
                nxt.append(g)
            except StopIteration:
                pass
        active = nxt


def _chain(*gens):
    for g in gens:
        yield from g


def build_kernel(apply_pad_mask: bool):
    nc = bacc.Bacc(
        "TRN2", target_bir_lowering=False, debug=False, num_devices=N_CORES
    )
    xT = nc.dram_tensor("xT", [C, B * T], BF16, kind="ExternalInput").ap()
    wqkv = nc.dram_tensor("wqkv", [C, 3 * HL * D], BF16, kind="ExternalInput").ap()
    wo = nc.dram_tensor("wo", [C, C], BF16, kind="ExternalInput").ap()
    tri = nc.dram_tensor("tri", [128, 128], BF16, kind="ExternalInput").ap()
    ident = nc.dram_tensor("ident", [128, 128], BF16, kind="ExternalInput").ap()
    padk = nc.dram_tensor("padk", [128, B * NK], BF16, kind="ExternalInput").ap()
    out = nc.dram_tensor("out", [512, C], F32, kind="ExternalOutput").ap()

    with tile.TileContext(nc) as tc:
        with (
            tc.tile_pool(name="const", bufs=1) as constp,
            tc.tile_pool(name="qk", bufs=1) as qkp,
            tc.tile_pool(name="vv", bufs=1) as vvp,
            tc.tile_pool(name="xw", bufs=1) as xwp,
            tc.tile_pool(name="work", bufs=2) as wk,
            tc.tile_pool(name="ytmp_pool", bufs=2) as ytp,
            tc.tile_pool(name="ps_ss", bufs=2, space="PSUM") as ps_ss,
            tc.tile_pool(name="ps_main", bufs=2, space="PSUM") as ps_main,
            tc.tile_pool(name="ps_y", bufs=1, space="PSUM") as ps_y,
            tc.tile_pool(name="dram", bufs=1, space="DRAM") as dram,
        ):
            # ---------------- constants ----------------
            tri_sb = constp.tile([128, 128], BF16, name="tri_sb")
            nc.sync.dma_start(tri_sb[:], tri[:])
            id_sb = constp.tile([128, 128], BF16, name="id_sb")
            nc.sync.dma_start(id_sb[:], ident[:])
            ones_f = constp.tile([65, 64], F32, name="ones_f")
            ones_sb = constp.tile([65, 64], F32R, name="ones_sb")
            for r in (0, 32, 64):
                nc.vector.memset(ones_f[r:r + 1, :], 1.0)
                nc.gpsimd.tensor_copy(ones_sb[r:r + 1, :], ones_f[r:r + 1, :])
            onesc_f = constp.tile([128, HL], F32, name="onesc_f")
            nc.vector.memset(onesc_f[:], 1.0)
            onesc = constp.tile([128, HL], BF16, name="onesc")
            nc.gpsimd.tensor_copy(onesc[:], onesc_f[:])
            if apply_pad_mask:
                padk_sb = constp.tile([128, B * NK], BF16, name="padk_sb")
                nc.sync.dma_start(padk_sb[:], padk[:])
            # warm the exp table before any real exp lands on ScalarE
            warm = constp.tile([1, 16], F32, name="warm")
            nc.vector.memset(warm[:], 0.0)
            nc.scalar.activation(warm[:], warm[:], AF.Exp)

            a2a_in = [dram.tile([N_CORES, 128, 256], BF16, name=f"a2a_in{b}")
                      for b in range(B)]
            a2a_out = [dram.tile([N_CORES, 128, 256], BF16, name=f"a2a_out{b}")
                       for b in range(B)]

            # weights first so the first matmul group is ready ASAP
            wqkv_sb = []
            for ct in range(NCT):
                w_sb = xwp.tile([128, 3 * HL * D], BF16, name=f"wqkv{ct}",
                                tag=f"wqkv{ct}")
                nc.sync.dma_start(w_sb[:], wqkv[ct * 128:(ct + 1) * 128, :])
                wqkv_sb.append(w_sb)

            qT = [None] * B
            kT = [None] * B
            V = [[None] * NK for _ in range(B)]
            ytn = [[None] * (B * NQ) for _ in range(HL)]

            def qkv_emit(b):
                """Projections for batch b, emitted as per-n triplets
                (k, q, v + the v transposes) so attention j=n can start as
                soon as triplet n has landed."""
                xt_sb = []
                for ct in range(NCT):
                    x_sb = xwp.tile([128, T], BF16, name=f"xt{b}_{ct}",
                                    tag=f"xt{b}_{ct}")
                    nc.sync.dma_start(
                        x_sb[:], xT[ct * 128:(ct + 1) * 128, b * T:(b + 1) * T]
                    )
                    xt_sb.append(x_sb)
                qT[b] = qkp.tile([128, T], BF16, name="qT", tag=f"qT{b}")
                kT[b] = qkp.tile([128, T], BF16, name="kT", tag=f"kT{b}")
                vT = qkp.tile([128, T], BF16, name="vT", tag=f"vT{b}")
                for n in range(NQ):
                    for which, dst in ((1, kT[b]), (0, qT[b]), (2, vT)):
                        p = ps_main.tile([128, 512], F32, name="p_mm",
                                         tag="ps")
                        for ct in range(NCT):
                            nc.tensor.matmul(
                                p[:],
                                wqkv_sb[ct][:, which * 128:(which + 1) * 128],
                                xt_sb[ct][:, n * 512:(n + 1) * 512],
                                start=(ct == 0),
                                stop=(ct == NCT - 1),
                            )
                        nc.vector.tensor_copy(dst[:, n * 512:(n + 1) * 512],
                                              p[:])
                        yield
                    for kt in range(4 * n, 4 * n + 4):
                        v_sb = vvp.tile([128, HL * 65], BF16, name=f"V{b}_{kt}",
                                        tag=f"V{b}_{kt}")
                        pt = ps_main.tile([128, 128], BF16, name="p_tr",
                                          tag="ps")
                        nc.tensor.transpose(pt[:],
                                            vT[:, kt * 128:(kt + 1) * 128],
                                            id_sb[:])
                        v3 = v_sb[:].rearrange("p (h e) -> p h e", h=HL)
                        nc.gpsimd.tensor_copy(v3[:, :, 64], onesc[:])
                        nc.vector.tensor_copy(
                            v3[:, :, 0:64],
                            pt[:].rearrange("p (h e) -> p h e", h=HL),
                        )
                        V[b][kt] = v_sb
                    yield

            def attn_emit(b):
                """Attention for batch b, both heads interleaved so their
                K=64 S^T matmuls run in different PE row-groups
                concurrently.  Yields per exp-block."""
                coll = [wk.tile([4 * NQ, 128], F32, name=f"coll{h}",
                                tag=f"coll{h}", bufs=2) for h in range(HL)]
                py = [None] * HL
                for j in range(NQ):
                    q0 = j * 512
                    for h in range(HL):
                        py[h] = ps_y.tile([65, 512], F32, name=f"p_y{h}",
                                          tag=f"py{h}")
                    n_kt = 4 * j + 4
                    # paired full blocks, then restricted diagonal singles
                    chunks = []
                    kt = 0
                    while kt < 4 * j:
                        chunks.append((kt, kt + 1))
                        kt += 2
                    for kt in range(4 * j, n_kt):
                        chunks.append((kt,))
                    for chunk in chunks:
                        pss = [None] * HL
                        lo = None
                        for h in range(HL):
                            h0 = h * 64
                            pss[h] = ps_ss.tile([128, 1024], F32, name="p_s",
                                                tag="pss")
                            for ci, kt in enumerate(chunk):
                                i = kt - 4 * j
                                off = 128 * i if i >= 0 else 0
                                base = 512 * ci
                                if lo is None:
                                    lo = base + off
                                nc.tensor.matmul(
                                    pss[h][:, base + off:base + 512],
                                    kT[b][h0:h0 + 64,
                                          kt * 128:(kt + 1) * 128],
                                    qT[b][h0:h0 + 64, q0 + off:q0 + 512],
                                    start=True,
                                    stop=True,
                                )
                        hi = 512 * (len(chunk) - 1) + 512
                        for h in range(HL):
                            p_sb = wk.tile([128, 1024], BF16, name="p_sb",
                                           tag="p_sb", bufs=4)
                            nc.scalar.activation(
                                p_sb[:, lo:hi], pss[h][:, lo:hi], AF.Exp,
                                scale=float(SCALE),
                            )
                            for ci, kt in enumerate(chunk):
                                i = kt - 4 * j
                                off = 128 * i if i >= 0 else 0
                                base = 512 * ci
                                if i >= 0:
                                    nc.vector.tensor_mul(
                                        p_sb[:, base + off:base + off + 128],
                                        p_sb[:, base + off:base + off + 128],
                                        tri_sb[:],
                                    )
                                if apply_pad_mask:
                                    nc.vector.tensor_scalar_mul(
                                        p_sb[:, base + off:base + 512],
                                        p_sb[:, base + off:base + 512],
                                        padk_sb[:, b * NK + kt:
                                                b * NK + kt + 1],
                                    )
                                nc.tensor.matmul(
                                    py[h][0:65, off:512],
                                    V[b][kt][:, h * 65:(h + 1) * 65],
                                    p_sb[:, base + off:base + 512],
                                    start=(kt == 0),
                                    stop=(kt == n_kt - 1),
                                )
                        yield
                    # evacuate PV accumulators
                    m = b * NQ + j
                    for h in range(HL):
                        yu = ytp.tile([64, 512], BF16, name=f"ytn{h}",
                                      tag=f"ytn{h}_{m}", bufs=1)
                        nc.vector.tensor_copy(yu[:], py[h][0:64, :])
                        ytn[h][m] = yu
                        srow = wk.tile([65, 512], F32, name=f"srow{h}",
                                       tag=f"srow{h}", bufs=2)
                        nc.vector.tensor_copy(srow[64:65, :], py[h][64:65, :])
                        nc.sync.dma_start(coll[h][4 * j:4 * j + 4, :],
                                          srow[64:65, :])
                    yield
                # wide reciprocals: all lanes busy, one per head
                for h in range(HL):
                    rcol = wk.tile([4 * NQ, 128], F32R, name=f"rcol{h}",
                                   tag=f"rcol{h}", bufs=2)
                    with nc.allow_low_precision(reason="fp32r softmax denom"):
                        nc.vector.reciprocal(rcol[:], coll[h][:])
                    for j in range(NQ):
                        m = b * NQ + j
                        rbase = 32 * (j % 3)
                        rr = wk.tile([65, 512], F32R, name=f"rrow{h}",
                                     tag=f"rr{h}_{j // 3}", bufs=2)
                        nc.sync.dma_start(rr[rbase:rbase + 1, :],
                                          rcol[4 * j:4 * j + 4, :])
                        pb = ps_main.tile([64, 512], F32, name="p_b", tag="ps")
                        nc.tensor.matmul(
                            pb[:], ones_sb[rbase:rbase + 1, :],
                            rr[rbase:rbase + 1, :], start=True, stop=True,
                        )
                        nc.vector.tensor_mul(ytn[h][m][:], ytn[h][m][:],
                                             pb[:])
                        # scatter the two 256-col halves into a2a slots
                        for half in range(2):
                            s = 2 * j + half
                            nc.sync.dma_start(
                                a2a_in[b][s, h * 64:(h + 1) * 64, :],
                                ytn[h][m][:, half * 256:(half + 1) * 256],
                            )
                        yield

            wo_sb = []
            ytf = [[None] * NCT for _ in range(B)]

            def wo_emit():
                # prefetch Wproj while batch-0 attention runs
                for ct in range(NCT):
                    w_sb = xwp.tile([128, C], BF16, name=f"wo{ct}",
                                    tag=f"wo{ct}")
                    nc.sync.dma_start(w_sb[:], wo[ct * 128:(ct + 1) * 128, :])
                    wo_sb.append(w_sb)
                    yield

            def proj_emit(b, delay):
                for _ in range(delay):
                    yield
                for s in range(NCT):
                    y_sb = xwp.tile([128, 256], BF16, name=f"ytf{b}_{s}",
                                    tag=f"ytf{b}_{s}")
                    nc.sync.dma_start(y_sb[:], a2a_out[b][s, :, :])
                    ytf[b][s] = y_sb
                    yield
                for mt in range(2):
                    o_sb = wk.tile([128, C], F32, name="o_sb", tag="o_sb")
                    for n in range(2):
                        po = ps_main.tile([128, 512], F32, name="p_o",
                                          tag="ps")
                        for ct in range(NCT):
                            nc.tensor.matmul(
                                po[:],
                                ytf[b][ct][:, mt * 128:(mt + 1) * 128],
                                wo_sb[ct][:, n * 512:(n + 1) * 512],
                                start=(ct == 0),
                                stop=(ct == NCT - 1),
                            )
                        nc.vector.tensor_copy(o_sb[:, n * 512:(n + 1) * 512],
                                              po[:])
                        yield
                    nc.sync.dma_start(
                        out[b * 256 + mt * 128:b * 256 + (mt + 1) * 128, :],
                        o_sb[:],
                    )

            # ---------------- emission schedule ----------------
            g_qkv0 = qkv_emit(0)
            for _ in range(4):          # k0, q0, v0, transposes 0-3
                next(g_qkv0)
            _drain(attn_emit(0), _chain(g_qkv0, qkv_emit(1), wo_emit()))
            nc.gpsimd.collective_compute(
                "AllToAll", mybir.AluOpType.bypass,
                replica_groups=[list(range(N_CORES))],
                ins=[a2a_in[0].opt()], outs=[a2a_out[0].opt()],
            )
            _drain(attn_emit(1), proj_emit(0, delay=10))
            nc.gpsimd.collective_compute(
                "AllToAll", mybir.AluOpType.bypass,
                replica_groups=[list(range(N_CORES))],
                ins=[a2a_in[1].opt()], outs=[a2a_out[1].opt()],
            )
            _drain(proj_emit(1, delay=0))

    nc.compile()
    return nc


def _host_inputs(x, tok_mask, Wqkv, Wproj, apply_pad_mask):
    x = np.ascontiguousarray(np.asarray(x, dtype=np.float32))
    Wqkv = np.ascontiguousarray(np.asarray(Wqkv, dtype=np.float32))
    Wproj = np.ascontiguousarray(np.asarray(Wproj, dtype=np.float32))
    bf = ml_dtypes.bfloat16
    xT = np.concatenate([x[b].T for b in range(B)], axis=1).astype(bf)
    wo_b = Wproj.astype(bf)
    r = np.arange(128)
    tri = (r[None, :] >= r[:, None]).astype(bf)  # keep if col >= row
    ident = np.eye(128, dtype=np.float32).astype(bf)
    if apply_pad_mask:
        padk = np.zeros((128, B * NK), np.float32)
        for b in range(B):
            padk[:, b * NK:(b + 1) * NK] = (
                np.asarray(tok_mask[b]).reshape(NK, 128).T.astype(np.float32)
            )
    else:
        padk = np.ones((128, B * NK), np.float32)
    padk = padk.astype(bf)

    in_maps = []
    for core in range(N_CORES):
        cols = slice(core * HL * D, (core + 1) * HL * D)
        wqkv_c = np.concatenate(
            [Wqkv[:, :C][:, cols], Wqkv[:, C:2 * C][:, cols],
             Wqkv[:, 2 * C:][:, cols]],
            axis=1,
        ).astype(bf)
        in_maps.append(
            {
                "xT": xT,
                "wqkv": wqkv_c,
                "wo": wo_b,
                "tri": tri,
                "ident": ident,
                "padk": padk,
            }
        )
    return in_maps


def kernel(x, tok_mask, Wqkv, Wproj, _run_kwargs=None):
    tok = np.asarray(tok_mask)
    apply_pad_mask = not bool(tok.all())
    key = apply_pad_mask
    if key not in _BUILD_CACHE:
        _BUILD_CACHE[key] = build_kernel(apply_pad_mask)
    nc = _BUILD_CACHE[key]
    in_maps = _host_inputs(x, tok_mask, Wqkv, Wproj, apply_pad_mask)
    kw = dict(_run_kwargs or {})
    res = bass_utils.run_bass_kernel_spmd(
        nc, in_maps, core_ids=list(range(N_CORES)), **kw
    )
    out = np.empty((B, T, C), np.float32)
    for core in range(N_CORES):
        o = res.results[core]["out"]
        for b in range(B):
            out[b, core * 256:(core + 1) * 256, :] = o[b * 256:(b + 1) * 256]
    kernel.last_result = res
    return out


# revision 6
# speedup vs baseline: 1.3727x; 1.1650x over previous
"""Multi-head causal self-attention on 8 Trainium2 NeuronCores (Bass/Tile).

Problem: y = proj(softmax(causal_mask(Q K^T / sqrt(D))) V) for B=2, T=2048,
C=1024, H=16 heads, D=64.

Sharding (tensor-parallel over heads, 8-way):
  - Core i owns heads {2i, 2i+1}: computes qT/kT/vT for its heads over both
    batches (full x, its 128-column slice of Wqkv) and runs causal attention
    per head fully on-core, producing normalized yT_local (head-dims on
    partitions, time on the free axis).
  - One 8-way AllToAll per batch reshards head-split -> time-split: core j
    ends with ytf[b] tiles [128, 256] covering time cols [256j, 256j+256)
    of batch b for all heads, and computes out rows = [b0 slice; b1 slice]
    via y @ Wproj.  Host concatenates the 8 col-slices per batch.

Processing is batch-major: attn(b=0) for BOTH heads (their K=64 S^T matmuls
land in different PE row-groups and run concurrently), then the b=0
AllToAll overlaps attn(b=1), and proj(b=0) overlaps the b=1 AllToAll, so
only a 0.5MB collective + half the projection remain on the tail.

All tensors are bf16 (fp32 PSUM accumulation); the softmax denominator
path (reciprocal + K=1 broadcast matmuls) stays fp32/fp32r.  Attention is
computed transposed (S^T[k, q], keys on partitions): no transposes in the
attention path, exp on ScalarE straight out of PSUM, and the denominator
comes free from a ones column appended to V (row 64 of the P@V
accumulator).  Causality is exact: S^T blocks strictly above the diagonal
are skipped, diagonal blocks use a restricted column range plus a
triangular multiplicative mask after exp.  Full blocks are computed in
1024-wide pairs so one ACTIVATE covers two blocks.

The kernel is emitted with interleaved instruction streams (generators):
batch-1 projections are woven into batch-0 attention and the b=0 output
projection into batch-1 attention, so the in-order PE queue always has
independent matmuls to run while ScalarE works through exp.
"""

import numpy as np
import ml_dtypes

import concourse.bass as bass
import concourse.mybir as mybir
import concourse.tile as tile
from concourse import bacc
from concourse import bass_utils

F32 = mybir.dt.float32
F32R = mybir.dt.float32r
BF16 = mybir.dt.bfloat16
AF = mybir.ActivationFunctionType

B, T, C = 2, 2048, 1024
H, D = 16, 64
N_CORES = 8
HL = H // N_CORES        # heads per core = 2
NCT = C // 128           # contraction tiles = 8
NQ = T // 512            # q tiles per batch = 4
NK = T // 128            # k tiles per batch = 16
SCALE = 1.0 / float(np.sqrt(D))  # 0.125

_BUILD_CACHE = {}


def _drain(*gens):
    """Round-robin the generators until all are exhausted."""
    active = list(gens)
    while active:
        nxt = []
        for g in active:
            try:
                next(g)
                nxt.append(g)
            except StopIteration:
                pass
        active = nxt


def _chain(*gens):
    for g in gens:
        yield from g


def build_kernel(apply_pad_mask: bool):
    nc = bacc.Bacc(
        "TRN2", target_bir_lowering=False, debug=False, num_devices=N_CORES
    )
    xT = nc.dram_tensor("xT", [C, B * T], BF16, kind="ExternalInput").ap()
    wqkv = nc.dram_tensor("wqkv", [C, 3 * HL * D], BF16, kind="ExternalInput").ap()
    wo = nc.dram_tensor("wo", [C, C], BF16, kind="ExternalInput").ap()
    tri = nc.dram_tensor("tri", [128, 128], BF16, kind="ExternalInput").ap()
    ident = nc.dram_tensor("ident", [128, 128], BF16, kind="ExternalInput").ap()
    padk = nc.dram_tensor("padk", [128, B * NK], BF16, kind="ExternalInput").ap()
    out = nc.dram_tensor("out", [512, C], F32, kind="ExternalOutput").ap()

    with tile.TileContext(nc) as tc:
        with (
            tc.tile_pool(name="const", bufs=1) as constp,
            tc.tile_pool(name="qk", bufs=1) as qkp,
            tc.tile_pool(name="vv", bufs=1) as vvp,
            tc.tile_pool(name="xw", bufs=1) as xwp,
            tc.tile_pool(name="work", bufs=2) as wk,
            tc.tile_pool(name="ytmp_pool", bufs=2) as ytp,
            tc.tile_pool(name="ps_ss", bufs=2, space="PSUM") as ps_ss,
            tc.tile_pool(name="ps_main", bufs=2, space="PSUM") as ps_main,
            tc.tile_pool(name="ps_y", bufs=1, space="PSUM") as ps_y,
            tc.tile_pool(name="dram", bufs=1, space="DRAM") as dram,
        ):
            # ---------------- constants ----------------
            tri_sb = constp.tile([128, 128], BF16, name="tri_sb")
            nc.sync.dma_start(tri_sb[:], tri[:])
            id_sb = constp.tile([128, 128], BF16, name="id_sb")
            nc.sync.dma_start(id_sb[:], ident[:])
            ones_f = constp.tile([65, 64], F32, name="ones_f")
            ones_sb = constp.tile([65, 64], F32R, name="ones_sb")
            for r in (0, 32, 64):
                nc.vector.memset(ones_f[r:r + 1, :], 1.0)
                nc.gpsimd.tensor_copy(ones_sb[r:r + 1, :], ones_f[r:r + 1, :])
            onesc_f = constp.tile([128, HL], F32, name="onesc_f")
            nc.vector.memset(onesc_f[:], 1.0)
            onesc = constp.tile([128, HL], BF16, name="onesc")
            nc.gpsimd.tensor_copy(onesc[:], onesc_f[:])
            if apply_pad_mask:
                padk_sb = constp.tile([128, B * NK], BF16, name="padk_sb")
                nc.sync.dma_start(padk_sb[:], padk[:])
            # warm the exp table before any real exp lands on ScalarE
            warm = constp.tile([1, 16], F32, name="warm")
            nc.vector.memset(warm[:], 0.0)
            nc.scalar.activation(warm[:], warm[:], AF.Exp)

            a2a_in = [dram.tile([N_CORES, 128, 256], BF16, name=f"a2a_in{b}")
                      for b in range(B)]
            a2a_out = [dram.tile([N_CORES, 128, 256], BF16, name=f"a2a_out{b}")
                       for b in range(B)]

            # weights first so the first matmul group is ready ASAP
            wqkv_sb = []
            for ct in range(NCT):
                w_sb = xwp.tile([128, 3 * HL * D], BF16, name=f"wqkv{ct}",
                                tag=f"wqkv{ct}")
                nc.sync.dma_start(w_sb[:], wqkv[ct * 128:(ct + 1) * 128, :])
                wqkv_sb.append(w_sb)

            qT = [None] * B
            kT = [None] * B
            V = [[None] * NK for _ in range(B)]
            ytn = [[None] * (B * NQ) for _ in range(HL)]

            def qkv_emit(b):
                """Projections for batch b, emitted as per-n triplets
                (k, q, v + the v transposes) so attention j=n can start as
                soon as triplet n has landed."""
                xt_sb = []
                for ct in range(NCT):
                    x_sb = xwp.tile([128, T], BF16, name=f"xt{b}_{ct}",
                                    tag=f"xt{b}_{ct}")
                    xt_sb.append(x_sb)
                # n-major 512-col chunks: the n=0 slices of every ct land
                # first so the first matmul group starts ~8us earlier
                for n in range(NQ):
                    for ct in range(NCT):
                        nc.sync.dma_start(
                            xt_sb[ct][:, n * 512:(n + 1) * 512],
                            xT[ct * 128:(ct + 1) * 128,
                               b * T + n * 512:b * T + (n + 1) * 512],
                        )
                qT[b] = qkp.tile([128, T], BF16, name="qT", tag=f"qT{b}")
                kT[b] = qkp.tile([128, T], BF16, name="kT", tag=f"kT{b}")
                vT = qkp.tile([128, T], BF16, name="vT", tag=f"vT{b}")
                for n in range(NQ):
                    for which, dst in ((1, kT[b]), (0, qT[b]), (2, vT)):
                        p = ps_main.tile([128, 512], F32, name="p_mm",
                                         tag="ps")
                        for ct in range(NCT):
                            nc.tensor.matmul(
                                p[:],
                                wqkv_sb[ct][:, which * 128:(which + 1) * 128],
                                xt_sb[ct][:, n * 512:(n + 1) * 512],
                                start=(ct == 0),
                                stop=(ct == NCT - 1),
                            )
                        nc.vector.tensor_copy(dst[:, n * 512:(n + 1) * 512],
                                              p[:])
                        yield
                    for kt in range(4 * n, 4 * n + 4):
                        v_sb = vvp.tile([128, HL * 65], BF16, name=f"V{b}_{kt}",
                                        tag=f"V{b}_{kt}")
                        pt = ps_main.tile([128, 128], BF16, name="p_tr",
                                          tag="ps")
                        nc.tensor.transpose(pt[:],
                                            vT[:, kt * 128:(kt + 1) * 128],
                                            id_sb[:])
                        v3 = v_sb[:].rearrange("p (h e) -> p h e", h=HL)
                        nc.gpsimd.tensor_copy(v3[:, :, 64], onesc[:])
                        nc.vector.tensor_copy(
                            v3[:, :, 0:64],
                            pt[:].rearrange("p (h e) -> p h e", h=HL),
                        )
                        V[b][kt] = v_sb
                    yield

            def attn_emit(b):
                """Attention for batch b, both heads interleaved so their
                K=64 S^T matmuls run in different PE row-groups
                concurrently.  Yields per exp-block."""
                coll = [wk.tile([4 * NQ, 128], F32, name=f"coll{h}",
                                tag=f"coll{h}", bufs=2) for h in range(HL)]
                py = [None] * HL
                for j in range(NQ):
                    q0 = j * 512
                    for h in range(HL):
                        py[h] = ps_y.tile([65, 512], F32, name=f"p_y{h}",
                                          tag=f"py{h}")
                    n_kt = 4 * j + 4
                    # paired full blocks, then restricted diagonal singles
                    chunks = []
                    kt = 0
                    while kt < 4 * j:
                        chunks.append((kt, kt + 1))
                        kt += 2
                    for kt in range(4 * j, n_kt):
                        chunks.append((kt,))
                    def make_pv(chunk, p_sbs, j, n_kt, pyl):
                        def emit():
                            for h in range(HL):
                                for ci, kt in enumerate(chunk):
                                    i = kt - 4 * j
                                    off = 128 * i if i >= 0 else 0
                                    base = 512 * ci
                                    nc.tensor.matmul(
                                        pyl[h][0:65, off:512],
                                        V[b][kt][:, h * 65:(h + 1) * 65],
                                        p_sbs[h][:, base + off:base + 512],
                                        start=(kt == 0),
                                        stop=(kt == n_kt - 1),
                                    )
                        return emit

                    pending = None
                    for chunk in chunks:
                        pss = [None] * HL
                        lo = None
                        for h in range(HL):
                            h0 = h * 64
                            pss[h] = ps_ss.tile([128, 1024], F32, name="p_s",
                                                tag="pss")
                            for ci, kt in enumerate(chunk):
                                i = kt - 4 * j
                                off = 128 * i if i >= 0 else 0
                                base = 512 * ci
                                if lo is None:
                                    lo = base + off
                                nc.tensor.matmul(
                                    pss[h][:, base + off:base + 512],
                                    kT[b][h0:h0 + 64,
                                          kt * 128:(kt + 1) * 128],
                                    qT[b][h0:h0 + 64, q0 + off:q0 + 512],
                                    start=True,
                                    stop=True,
                                )
                        hi = 512 * (len(chunk) - 1) + 512
                        p_sbs = [None] * HL
                        for h in range(HL):
                            p_sbs[h] = wk.tile([128, 1024], BF16, name="p_sb",
                                               tag="p_sb", bufs=6)
                            nc.scalar.activation(
                                p_sbs[h][:, lo:hi], pss[h][:, lo:hi], AF.Exp,
                                scale=float(SCALE),
                            )
                            for ci, kt in enumerate(chunk):
                                i = kt - 4 * j
                                off = 128 * i if i >= 0 else 0
                                base = 512 * ci
                                if i >= 0:
                                    nc.vector.tensor_mul(
                                        p_sbs[h][:, base + off:
                                                 base + off + 128],
                                        p_sbs[h][:, base + off:
                                                 base + off + 128],
                                        tri_sb[:],
                                    )
                                if apply_pad_mask:
                                    nc.vector.tensor_scalar_mul(
                                        p_sbs[h][:, base + off:base + 512],
                                        p_sbs[h][:, base + off:base + 512],
                                        padk_sb[:, b * NK + kt:
                                                b * NK + kt + 1],
                                    )
                        # PV of the PREVIOUS chunk: keeps exp + tri-mul off
                        # the in-order PE queue's critical path
                        if pending is not None:
                            pending()
                        pending = make_pv(chunk, p_sbs, j, n_kt, list(py))
                        yield
                    if pending is not None:
                        pending()
                    # evacuate PV accumulators
                    m = b * NQ + j
                    for h in range(HL):
                        yu = ytp.tile([64, 512], BF16, name=f"ytn{h}",
                                      tag=f"ytn{h}_{m}", bufs=1)
                        nc.vector.tensor_copy(yu[:], py[h][0:64, :])
                        ytn[h][m] = yu
                        srow = wk.tile([65, 512], F32, name=f"srow{h}",
                                       tag=f"srow{h}", bufs=2)
                        nc.vector.tensor_copy(srow[64:65, :], py[h][64:65, :])
                        nc.sync.dma_start(coll[h][4 * j:4 * j + 4, :],
                                          srow[64:65, :])
                    yield
                # wide reciprocals: all lanes busy, one per head
                for h in range(HL):
                    rcol = wk.tile([4 * NQ, 128], F32R, name=f"rcol{h}",
                                   tag=f"rcol{h}", bufs=2)
                    with nc.allow_low_precision(reason="fp32r softmax denom"):
                        nc.vector.reciprocal(rcol[:], coll[h][:])
                    for j in range(NQ):
                        m = b * NQ + j
                        rbase = 32 * (j % 3)
                        rr = wk.tile([65, 512], F32R, name=f"rrow{h}",
                                     tag=f"rr{h}_{j // 3}", bufs=2)
                        nc.sync.dma_start(rr[rbase:rbase + 1, :],
                                          rcol[4 * j:4 * j + 4, :])
                        pb = ps_main.tile([64, 512], F32, name="p_b", tag="ps")
                        nc.tensor.matmul(
                            pb[:], ones_sb[rbase:rbase + 1, :],
                            rr[rbase:rbase + 1, :], start=True, stop=True,
                        )
                        nc.vector.tensor_mul(ytn[h][m][:], ytn[h][m][:],
                                             pb[:])
                        # scatter the two 256-col halves into a2a slots
                        for half in range(2):
                            s = 2 * j + half
                            nc.sync.dma_start(
                                a2a_in[b][s, h * 64:(h + 1) * 64, :],
                                ytn[h][m][:, half * 256:(half + 1) * 256],
                            )
                        yield

            wo_sb = []
            ytf = [[None] * NCT for _ in range(B)]

            def wo_emit():
                # prefetch Wproj while batch-0 attention runs
                for ct in range(NCT):
                    w_sb = xwp.tile([128, C], BF16, name=f"wo{ct}",
                                    tag=f"wo{ct}")
                    nc.sync.dma_start(w_sb[:], wo[ct * 128:(ct + 1) * 128, :])
                    wo_sb.append(w_sb)
                    yield

            def proj_emit(b, delay):
                for _ in range(delay):
                    yield
                for s in range(NCT):
                    y_sb = xwp.tile([128, 256], BF16, name=f"ytf{b}_{s}",
                                    tag=f"ytf{b}_{s}")
                    nc.sync.dma_start(y_sb[:], a2a_out[b][s, :, :])
                    ytf[b][s] = y_sb
                    yield
                for mt in range(2):
                    o_sb = wk.tile([128, C], F32, name="o_sb", tag="o_sb")
                    for n in range(2):
                        po = ps_main.tile([128, 512], F32, name="p_o",
                                          tag="ps")
                        for ct in range(NCT):
                            nc.tensor.matmul(
                                po[:],
                                ytf[b][ct][:, mt * 128:(mt + 1) * 128],
                                wo_sb[ct][:, n * 512:(n + 1) * 512],
                                start=(ct == 0),
                                stop=(ct == NCT - 1),
                            )
                        nc.vector.tensor_copy(o_sb[:, n * 512:(n + 1) * 512],
                                              po[:])
                        yield
                    nc.sync.dma_start(
                        out[b * 256 + mt * 128:b * 256 + (mt + 1) * 128, :],
                        o_sb[:],
                    )

            # ---------------- emission schedule ----------------
            g_qkv0 = qkv_emit(0)
            for _ in range(4):          # k0, q0, v0, transposes 0-3
                next(g_qkv0)
            _drain(attn_emit(0), _chain(g_qkv0, qkv_emit(1), wo_emit()))
            nc.gpsimd.collective_compute(
                "AllToAll", mybir.AluOpType.bypass,
                replica_groups=[list(range(N_CORES))],
                ins=[a2a_in[0].opt()], outs=[a2a_out[0].opt()],
            )
            _drain(attn_emit(1), proj_emit(0, delay=10))
            nc.gpsimd.collective_compute(
                "AllToAll", mybir.AluOpType.bypass,
                replica_groups=[list(range(N_CORES))],
                ins=[a2a_in[1].opt()], outs=[a2a_out[1].opt()],
            )
            _drain(proj_emit(1, delay=0))

    nc.compile()
    return nc


def _host_inputs(x, tok_mask, Wqkv, Wproj, apply_pad_mask):
    x = np.ascontiguousarray(np.asarray(x, dtype=np.float32))
    Wqkv = np.ascontiguousarray(np.asarray(Wqkv, dtype=np.float32))
    Wproj = np.ascontiguousarray(np.asarray(Wproj, dtype=np.float32))
    bf = ml_dtypes.bfloat16
    xT = np.concatenate([x[b].T for b in range(B)], axis=1).astype(bf)
    wo_b = Wproj.astype(bf)
    r = np.arange(128)
    tri = (r[None, :] >= r[:, None]).astype(bf)  # keep if col >= row
    ident = np.eye(128, dtype=np.float32).astype(bf)
    if apply_pad_mask:
        padk = np.zeros((128, B * NK), np.float32)
        for b in range(B):
            padk[:, b * NK:(b + 1) * NK] = (
                np.asarray(tok_mask[b]).reshape(NK, 128).T.astype(np.float32)
            )
    else:
        padk = np.ones((128, B * NK), np.float32)
    padk = padk.astype(bf)

    in_maps = []
    for core in range(N_CORES):
        cols = slice(core * HL * D, (core + 1) * HL * D)
        wqkv_c = np.concatenate(
            [Wqkv[:, :C][:, cols], Wqkv[:, C:2 * C][:, cols],
             Wqkv[:, 2 * C:][:, cols]],
            axis=1,
        ).astype(bf)
        in_maps.append(
            {
                "xT": xT,
                "wqkv": wqkv_c,
                "wo": wo_b,
                "tri": tri,
                "ident": ident,
                "padk": padk,
            }
        )
    return in_maps


def kernel(x, tok_mask, Wqkv, Wproj, _run_kwargs=None):
    tok = np.asarray(tok_mask)
    apply_pad_mask = not bool(tok.all())
    key = apply_pad_mask
    if key not in _BUILD_CACHE:
        _BUILD_CACHE[key] = build_kernel(apply_pad_mask)
    nc = _BUILD_CACHE[key]
    in_maps = _host_inputs(x, tok_mask, Wqkv, Wproj, apply_pad_mask)
    kw = dict(_run_kwargs or {})
    res = bass_utils.run_bass_kernel_spmd(
        nc, in_maps, core_ids=list(range(N_CORES)), **kw
    )
    out = np.empty((B, T, C), np.float32)
    for core in range(N_CORES):
        o = res.results[core]["out"]
        for b in range(B):
            out[b, core * 256:(core + 1) * 256, :] = o[b * 256:(b + 1) * 256]
    kernel.last_result = res
    return out


# revision 15
# speedup vs baseline: 1.3939x; 1.0154x over previous
"""Multi-head causal self-attention on 8 Trainium2 NeuronCores (Bass/Tile).

Problem: y = proj(softmax(causal_mask(Q K^T / sqrt(D))) V) for B=2, T=2048,
C=1024, H=16 heads, D=64.

Sharding (tensor-parallel over heads, 8-way):
  - Core i owns heads {2i, 2i+1}: computes qT/kT/vT for its heads over both
    batches (full x, its 128-column slice of Wqkv) and runs causal attention
    per head fully on-core, producing normalized yT_local (head-dims on
    partitions, time on the free axis).
  - One 8-way AllToAll per batch reshards head-split -> time-split: core j
    ends with ytf[b] tiles [128, 256] covering time cols [256j, 256j+256)
    of batch b for all heads, and computes out rows = [b0 slice; b1 slice]
    via y @ Wproj.  Host concatenates the 8 col-slices per batch.

Processing is batch-major: attn(b=0) for BOTH heads (their K=64 S^T matmuls
land in different PE row-groups and run concurrently), then the b=0
AllToAll overlaps attn(b=1), and proj(b=0) overlaps the b=1 AllToAll, so
only a 0.5MB collective + half the projection remain on the tail.

All tensors are bf16 (fp32 PSUM accumulation); the softmax denominator
path (reciprocal + K=1 broadcast matmuls) stays fp32/fp32r.  Attention is
computed transposed (S^T[k, q], keys on partitions): no transposes in the
attention path, exp on ScalarE straight out of PSUM, and the denominator
comes free from a ones column appended to V (row 64 of the P@V
accumulator).  Causality is exact: S^T blocks strictly above the diagonal
are skipped, diagonal blocks use a restricted column range plus a
triangular multiplicative mask after exp.  Full blocks are computed in
1024-wide pairs so one ACTIVATE covers two blocks.

The kernel is emitted with interleaved instruction streams (generators):
batch-1 projections are woven into batch-0 attention and the b=0 output
projection into batch-1 attention, so the in-order PE queue always has
independent matmuls to run while ScalarE works through exp.
"""

import numpy as np
import ml_dtypes

import concourse.bass as bass
import concourse.mybir as mybir
import concourse.tile as tile
from concourse import bacc
from concourse import bass_utils

F32 = mybir.dt.float32
F32R = mybir.dt.float32r
BF16 = mybir.dt.bfloat16
AF = mybir.ActivationFunctionType

B, T, C = 2, 2048, 1024
H, D = 16, 64
N_CORES = 8
HL = H // N_CORES        # heads per core = 2
NCT = C // 128           # contraction tiles = 8
NQ = T // 512            # q tiles per batch = 4
NK = T // 128            # k tiles per batch = 16
SCALE = 1.0 / float(np.sqrt(D))  # 0.125

_BUILD_CACHE = {}


def _drain(*gens, turns=None):
    """Round-robin the generators until all are exhausted.  turns[i] gives
    generator i that many next() calls per round (default 1)."""
    active = list(gens)
    tmap = {id(g): (turns[i] if turns else 1) for i, g in enumerate(gens)}
    while active:
        nxt = []
        for g in active:
            alive = True
            for _ in range(tmap[id(g)]):
                try:
                    next(g)
                except StopIteration:
                    alive = False
                    break
            if alive:
                nxt.append(g)
        active = nxt


def _chain(*gens):
    for g in gens:
        yield from g


def build_kernel(apply_pad_mask: bool):
    nc = bacc.Bacc(
        "TRN2", target_bir_lowering=False, debug=False, num_devices=N_CORES
    )
    xT = nc.dram_tensor("xT", [C, B * T], BF16, kind="ExternalInput").ap()
    wqkv = nc.dram_tensor("wqkv", [C, 3 * HL * D], BF16, kind="ExternalInput").ap()
    wo = nc.dram_tensor("wo", [C, C], BF16, kind="ExternalInput").ap()
    tri = nc.dram_tensor("tri", [128, 128], BF16, kind="ExternalInput").ap()
    ident = nc.dram_tensor("ident", [128, 128], BF16, kind="ExternalInput").ap()
    padk = nc.dram_tensor("padk", [128, B * NK], BF16, kind="ExternalInput").ap()
    out = nc.dram_tensor("out", [512, C], F32, kind="ExternalOutput").ap()

    with tile.TileContext(nc) as tc:
        with (
            tc.tile_pool(name="const", bufs=1) as constp,
            tc.tile_pool(name="qk", bufs=1) as qkp,
            tc.tile_pool(name="vv", bufs=1) as vvp,
            tc.tile_pool(name="xw", bufs=1) as xwp,
            tc.tile_pool(name="work", bufs=2) as wk,
            tc.tile_pool(name="ytmp_pool", bufs=2) as ytp,
            tc.tile_pool(name="ps_ss", bufs=2, space="PSUM") as ps_ss,
            tc.tile_pool(name="ps_main", bufs=2, space="PSUM") as ps_main,
            tc.tile_pool(name="ps_y", bufs=1, space="PSUM") as ps_y,
            tc.tile_pool(name="dram", bufs=1, space="DRAM") as dram,
        ):
            # -------- critical-path DMAs first: x(b=0) n=0 + wqkv --------
            xt = {}
            for b in range(B):
                xt[b] = [xwp.tile([128, T], BF16, name=f"xt{b}_{ct}",
                                  tag=f"xt{b}_{ct}") for ct in range(NCT)]
            for ct in range(NCT):
                nc.sync.dma_start(xt[0][ct][:, 0:512],
                                  xT[ct * 128:(ct + 1) * 128, 0:512])
            wqkv_sb = []
            for ct in range(NCT):
                w_sb = xwp.tile([128, 3 * HL * D], BF16, name=f"wqkv{ct}",
                                tag=f"wqkv{ct}")
                nc.sync.dma_start(w_sb[:], wqkv[ct * 128:(ct + 1) * 128, :])
                wqkv_sb.append(w_sb)

            # ---------------- constants ----------------
            tri_sb = constp.tile([128, 128], BF16, name="tri_sb")
            nc.sync.dma_start(tri_sb[:], tri[:])
            id_sb = constp.tile([128, 128], BF16, name="id_sb")
            nc.sync.dma_start(id_sb[:], ident[:])
            ones_f = constp.tile([65, 64], F32, name="ones_f")
            ones_sb = constp.tile([65, 64], F32R, name="ones_sb")
            for r in (0, 32, 64):
                nc.vector.memset(ones_f[r:r + 1, :], 1.0)
                nc.gpsimd.tensor_copy(ones_sb[r:r + 1, :], ones_f[r:r + 1, :])
            onesc_f = constp.tile([128, HL], F32, name="onesc_f")
            nc.vector.memset(onesc_f[:], 1.0)
            onesc = constp.tile([128, HL], BF16, name="onesc")
            nc.gpsimd.tensor_copy(onesc[:], onesc_f[:])
            if apply_pad_mask:
                padk_sb = constp.tile([128, B * NK], BF16, name="padk_sb")
                nc.sync.dma_start(padk_sb[:], padk[:])
            # warm the exp table before any real exp lands on ScalarE
            warm = constp.tile([1, 16], F32, name="warm")
            nc.vector.memset(warm[:], 0.0)
            nc.scalar.activation(warm[:], warm[:], AF.Exp)

            a2a_in = [dram.tile([N_CORES, 128, 256], BF16, name=f"a2a_in{b}")
                      for b in range(B)]
            a2a_out = [dram.tile([N_CORES, 128, 256], BF16, name=f"a2a_out{b}")
                       for b in range(B)]

            qT = [None] * B
            kT = [None] * B
            V = [[None] * NK for _ in range(B)]
            ytn = [[None] * (B * NQ) for _ in range(HL)]

            def qkv_emit(b):
                """Projections for batch b, emitted as per-n triplets
                (k, q, v + the v transposes) so attention j=n can start as
                soon as triplet n has landed.  Yields are ~1us quanta so
                interleaved attention chunks are not delayed long."""
                xt_sb = xt[b]
                # n-major 512-col chunks: the n=0 slices of every ct land
                # first so the first matmul group starts earlier.  b=0 n=0
                # was already issued at kernel start.
                for n in range(NQ):
                    for ct in range(NCT):
                        if b == 0 and n == 0:
                            continue
                        nc.sync.dma_start(
                            xt_sb[ct][:, n * 512:(n + 1) * 512],
                            xT[ct * 128:(ct + 1) * 128,
                               b * T + n * 512:b * T + (n + 1) * 512],
                        )
                qT[b] = qkp.tile([128, T], BF16, name="qT", tag=f"qT{b}")
                kT[b] = qkp.tile([128, T], BF16, name="kT", tag=f"kT{b}")
                vT = qkp.tile([128, T], BF16, name="vT", tag=f"vT{b}")
                for n in range(NQ):
                    for which, dst in ((1, kT[b]), (0, qT[b]), (2, vT)):
                        p = ps_main.tile([128, 512], F32, name="p_mm",
                                         tag="ps")
                        for ct in range(NCT):
                            nc.tensor.matmul(
                                p[:],
                                wqkv_sb[ct][:, which * 128:(which + 1) * 128],
                                xt_sb[ct][:, n * 512:(n + 1) * 512],
                                start=(ct == 0),
                                stop=(ct == NCT - 1),
                            )
                            if ct == 3:
                                yield
                        nc.vector.tensor_copy(dst[:, n * 512:(n + 1) * 512],
                                              p[:])
                        yield
                    for kt in range(4 * n, 4 * n + 4):
                        v_sb = vvp.tile([128, HL * 65], BF16, name=f"V{b}_{kt}",
                                        tag=f"V{b}_{kt}")
                        pt = ps_main.tile([128, 128], BF16, name="p_tr",
                                          tag="ps")
                        nc.tensor.transpose(pt[:],
                                            vT[:, kt * 128:(kt + 1) * 128],
                                            id_sb[:])
                        v3 = v_sb[:].rearrange("p (h e) -> p h e", h=HL)
                        nc.gpsimd.tensor_copy(v3[:, :, 64], onesc[:])
                        nc.vector.tensor_copy(
                            v3[:, :, 0:64],
                            pt[:].rearrange("p (h e) -> p h e", h=HL),
                        )
                        V[b][kt] = v_sb
                        if kt % 2 == 1:
                            yield

            def attn_emit(b):
                """Attention for batch b, both heads interleaved so their
                K=64 S^T matmuls run in different PE row-groups
                concurrently.  Yields per exp-block."""
                coll = [[None] * NQ for _ in range(HL)]
                py = [None] * HL
                for j in range(NQ):
                    q0 = j * 512
                    for h in range(HL):
                        py[h] = ps_y.tile([65, 512], F32, name=f"p_y{h}",
                                          tag=f"py{h}")
                    n_kt = 4 * j + 4
                    # paired full blocks, then restricted diagonal singles
                    chunks = []
                    kt = 0
                    while kt < 4 * j:
                        chunks.append((kt, kt + 1))
                        kt += 2
                    for kt in range(4 * j, n_kt):
                        chunks.append((kt,))
                    def make_pv(chunk, p_sbs, j, n_kt, pyl):
                        def emit():
                            for h in range(HL):
                                for ci, kt in enumerate(chunk):
                                    i = kt - 4 * j
                                    off = 128 * i if i >= 0 else 0
                                    base = 512 * ci
                                    nc.tensor.matmul(
                                        pyl[h][0:65, off:512],
                                        V[b][kt][:, h * 65:(h + 1) * 65],
                                        p_sbs[h][:, base + off:base + 512],
                                        start=(kt == 0),
                                        stop=(kt == n_kt - 1),
                                    )
                        return emit

                    pending = None
                    for chunk in chunks:
                        pss = [None] * HL
                        lo = None
                        for h in range(HL):
                            h0 = h * 64
                            pss[h] = ps_ss.tile([128, 1024], F32, name="p_s",
                                                tag="pss")
                            for ci, kt in enumerate(chunk):
                                i = kt - 4 * j
                                off = 128 * i if i >= 0 else 0
                                base = 512 * ci
                                if lo is None:
                                    lo = base + off
                                nc.tensor.matmul(
                                    pss[h][:, base + off:base + 512],
                                    kT[b][h0:h0 + 64,
                                          kt * 128:(kt + 1) * 128],
                                    qT[b][h0:h0 + 64, q0 + off:q0 + 512],
                                    start=True,
                                    stop=True,
                                )
                        hi = 512 * (len(chunk) - 1) + 512
                        p_sbs = [None] * HL
                        for h in range(HL):
                            p_sbs[h] = wk.tile([128, 1024], BF16, name="p_sb",
                                               tag="p_sb", bufs=6)
                            nc.scalar.activation(
                                p_sbs[h][:, lo:hi], pss[h][:, lo:hi], AF.Exp,
                                scale=float(SCALE),
                            )
                            for ci, kt in enumerate(chunk):
                                i = kt - 4 * j
                                off = 128 * i if i >= 0 else 0
                                base = 512 * ci
                                if i >= 0:
                                    nc.vector.tensor_mul(
                                        p_sbs[h][:, base + off:
                                                 base + off + 128],
                                        p_sbs[h][:, base + off:
                                                 base + off + 128],
                                        tri_sb[:],
                                    )
                                if apply_pad_mask:
                                    nc.vector.tensor_scalar_mul(
                                        p_sbs[h][:, base + off:base + 512],
                                        p_sbs[h][:, base + off:base + 512],
                                        padk_sb[:, b * NK + kt:
                                                b * NK + kt + 1],
                                    )
                        # PV of the PREVIOUS chunk: keeps exp + tri-mul off
                        # the in-order PE queue's critical path
                        if pending is not None:
                            pending()
                        pending = make_pv(chunk, p_sbs, j, n_kt, list(py))
                        yield
                    if pending is not None:
                        pending()
                    # evacuate PV accumulators + normalize this j in place:
                    # the whole chain overlaps the next j's compute, so only
                    # j=3's chain precedes the collective trigger
                    m = b * NQ + j
                    rbase = 32 * (j % 3)
                    for h in range(HL):
                        yu = ytp.tile([64, 512], BF16, name=f"ytn{h}",
                                      tag=f"ytn{h}_{m}", bufs=1)
                        nc.vector.tensor_copy(yu[:], py[h][0:64, :])
                        ytn[h][m] = yu
                        srow = wk.tile([65, 512], F32, name=f"srow{h}",
                                       tag=f"srow{h}", bufs=2)
                        nc.vector.tensor_copy(srow[64:65, :], py[h][64:65, :])
                        scol = wk.tile([4, 128], F32, name=f"scol{h}",
                                       tag=f"scol{h}", bufs=2)
                        nc.sync.dma_start(scol[:], srow[64:65, :])
                        coll[h][j] = scol
                    yield
                    for h in range(HL):
                        rcol = wk.tile([4, 128], F32R, name=f"rcol{h}",
                                       tag=f"rcol{h}", bufs=2)
                        with nc.allow_low_precision(
                                reason="fp32r softmax denom"):
                            nc.vector.reciprocal(rcol[:], coll[h][j][:])
                        rr = wk.tile([65, 512], F32R, name=f"rrow{h}",
                                     tag=f"rr{h}_{j // 3}", bufs=2)
                        nc.sync.dma_start(rr[rbase:rbase + 1, :],
                                          rcol[:])
                        # pb reuses the (just-evacuated) py slot of this head
                        pb = ps_y.tile([64, 512], F32, name=f"p_b{h}",
                                       tag=f"py{h}")
                        nc.tensor.matmul(
                            pb[:], ones_sb[rbase:rbase + 1, :],
                            rr[rbase:rbase + 1, :], start=True, stop=True,
                        )
                        nc.vector.tensor_mul(ytn[h][m][:], ytn[h][m][:],
                                             pb[:])
                        # scatter the two 256-col halves into a2a slots
                        for half in range(2):
                            s = 2 * j + half
                            nc.sync.dma_start(
                                a2a_in[b][s, h * 64:(h + 1) * 64, :],
                                ytn[h][m][:, half * 256:(half + 1) * 256],
                            )
                    yield

            wo_sb = []
            ytf = [[None] * NCT for _ in range(B)]

            def wo_emit():
                # prefetch Wproj while batch-0 attention runs
                for ct in range(NCT):
                    w_sb = xwp.tile([128, C], BF16, name=f"wo{ct}",
                                    tag=f"wo{ct}")
                    nc.sync.dma_start(w_sb[:], wo[ct * 128:(ct + 1) * 128, :])
                    wo_sb.append(w_sb)
                    yield

            def proj_emit(b, delay):
                for _ in range(delay):
                    yield
                for s in range(NCT):
                    y_sb = xwp.tile([128, 256], BF16, name=f"ytf{b}_{s}",
                                    tag=f"ytf{b}_{s}")
                    nc.sync.dma_start(y_sb[:], a2a_out[b][s, :, :])
                    ytf[b][s] = y_sb
                    yield
                for mt in range(2):
                    o_sb = wk.tile([128, C], F32, name="o_sb", tag="o_sb")
                    for n in range(2):
                        po = ps_main.tile([128, 512], F32, name="p_o",
                                          tag="ps")
                        for ct in range(NCT):
                            nc.tensor.matmul(
                                po[:],
                                ytf[b][ct][:, mt * 128:(mt + 1) * 128],
                                wo_sb[ct][:, n * 512:(n + 1) * 512],
                                start=(ct == 0),
                                stop=(ct == NCT - 1),
                            )
                            if ct == 3:
                                yield
                        nc.vector.tensor_copy(o_sb[:, n * 512:(n + 1) * 512],
                                              po[:])
                        yield
                    nc.sync.dma_start(
                        out[b * 256 + mt * 128:b * 256 + (mt + 1) * 128, :],
                        o_sb[:],
                    )

            # ---------------- emission schedule ----------------
            g_qkv0 = qkv_emit(0)
            for _ in range(8):          # k0, q0, v0, transposes 0-3
                next(g_qkv0)
            _drain(attn_emit(0), _chain(g_qkv0, qkv_emit(1), wo_emit()),
                   turns=[1, 2])
            nc.gpsimd.collective_compute(
                "AllToAll", mybir.AluOpType.bypass,
                replica_groups=[list(range(N_CORES))],
                ins=[a2a_in[0].opt()], outs=[a2a_out[0].opt()],
            )
            _drain(attn_emit(1), proj_emit(0, delay=10))
            nc.gpsimd.collective_compute(
                "AllToAll", mybir.AluOpType.bypass,
                replica_groups=[list(range(N_CORES))],
                ins=[a2a_in[1].opt()], outs=[a2a_out[1].opt()],
            )
            _drain(proj_emit(1, delay=0))

    nc.compile()
    return nc


def _host_inputs(x, tok_mask, Wqkv, Wproj, apply_pad_mask):
    x = np.ascontiguousarray(np.asarray(x, dtype=np.float32))
    Wqkv = np.ascontiguousarray(np.asarray(Wqkv, dtype=np.float32))
    Wproj = np.ascontiguousarray(np.asarray(Wproj, dtype=np.float32))
    bf = ml_dtypes.bfloat16
    xT = np.concatenate([x[b].T for b in range(B)], axis=1).astype(bf)
    wo_b = Wproj.astype(bf)
    r = np.arange(128)
    tri = (r[None, :] >= r[:, None]).astype(bf)  # keep if col >= row
    ident = np.eye(128, dtype=np.float32).astype(bf)
    if apply_pad_mask:
        padk = np.zeros((128, B * NK), np.float32)
        for b in range(B):
            padk[:, b * NK:(b + 1) * NK] = (
                np.asarray(tok_mask[b]).reshape(NK, 128).T.astype(np.float32)
            )
    else:
        padk = np.ones((128, B * NK), np.float32)
    padk = padk.astype(bf)

    in_maps = []
    for core in range(N_CORES):
        cols = slice(core * HL * D, (core + 1) * HL * D)
        wqkv_c = np.concatenate(
            [Wqkv[:, :C][:, cols], Wqkv[:, C:2 * C][:, cols],
             Wqkv[:, 2 * C:][:, cols]],
            axis=1,
        ).astype(bf)
        in_maps.append(
            {
                "xT": xT,
                "wqkv": wqkv_c,
                "wo": wo_b,
                "tri": tri,
                "ident": ident,
                "padk": padk,
            }
        )
    return in_maps


def kernel(x, tok_mask, Wqkv, Wproj, _run_kwargs=None):
    tok = np.asarray(tok_mask)
    apply_pad_mask = not bool(tok.all())
    key = apply_pad_mask
    if key not in _BUILD_CACHE:
        _BUILD_CACHE[key] = build_kernel(apply_pad_mask)
    nc = _BUILD_CACHE[key]
    in_maps = _host_inputs(x, tok_mask, Wqkv, Wproj, apply_pad_mask)
    kw = dict(_run_kwargs or {})
    res = bass_utils.run_bass_kernel_spmd(
        nc, in_maps, core_ids=list(range(N_CORES)), **kw
    )
    out = np.empty((B, T, C), np.float32)
    for core in range(N_CORES):
        o = res.results[core]["out"]
        for b in range(B):
            out[b, core * 256:(core + 1) * 256, :] = o[b * 256:(b + 1) * 256]
    kernel.last_result = res
    return out
